# revision 25
# baseline (speedup 1.0000x reference)
"""MixtureOfAttention forward for Trainium2 (8 NeuronCores, data-parallel over B).

Math (exactly equivalent to the reference):
  s_b   = rsqrt(mean(x_b^2) + eps)                      (per token)
  logits= s * (x @ (diag(norm_w) @ router_w)) + router_b
  r     = softmax(logits)                                [B, 4]
  y     = x + sum_e (r_e * s) * (x_e @ W_e) + r @ C
  W_e   = diag(norm_w_e) @ Wv_e @ proj_w_e @ out_w_e     [512, 2048]  (host-folded)
  C_e   = proj_b_e @ out_w_e                             [2048]       (host-folded)
(The seq_len==1 attention is the identity on v, so only the v-slice of qkv_w
participates.  The r @ C term is applied on host from the device-computed
routing probs; it is exactly zero for proj_b == 0.)

Implementation: all GEMMs run in fp8-e4m3 with DoubleRow perf mode
(contraction 256/matmul, ~1.8x fp32r throughput).  x is pre-transposed and
pre-quantized on host (hi + lo residual); the router consumes hi/lo for
near-exact logits, the main GEMM consumes hi only.  Scales: x*2^4, W*2^7,
residuals *2^4 further; the combined 2^-11 is folded into the rms scale s.
"""

import sys

sys.path.insert(0, "/opt/trn_rl_repo")

import numpy as np
import ml_dtypes

import concourse.bass as bass
import concourse.bacc as bacc
import concourse.mybir as mybir
import concourse.tile as tile
from concourse import bass_utils, masks

F8 = ml_dtypes.float8_e4m3

B, D, E = 32768, 2048, 4
dE = D // E  # 512
EPS = 1e-6
N_CORES = 8
P = 128
BC = B // N_CORES  # 4096 tokens per core
NT = BC // P  # 32 tiles per core
KC = D // P  # 16 feature chunks
NDR = KC // 2  # 8 DoubleRow chunks (256 features each)
NJ = 4  # output chunks of 512
RWPAD = 16  # router weight free-dim pad (16B alignment for the DR pair stride)

SX = 16.0  # x scale before fp8
SW = 128.0  # W / router_w scale before fp8
S_SCALE = float(2.0**22)  # folds 2^-11 (=1/(SX*SW)) into rec = 1/sqrt(...)

_dt = mybir.dt
AF = mybir.ActivationFunctionType
ALU = mybir.AluOpType
DR = mybir.MatmulPerfMode.DoubleRow


def build(nt: int, skip_rt: bool = False, rb_zero: bool = False):
    bc = nt * P
    nc = bacc.Bacc("TRN2", target_bir_lowering=False, debug=False, num_devices=N_CORES)

    x_d = nc.dram_tensor("x", [bc, D], _dt.float32, kind="ExternalInput")
    xhi_d = nc.dram_tensor("xhi", [nt, P, D], _dt.float8e4, kind="ExternalInput")
    xlo_d = nc.dram_tensor("xlo", [nt, P, D], _dt.float8e4, kind="ExternalInput")
    w_d = nc.dram_tensor("w8", [P, NDR, 2, D], _dt.float8e4, kind="ExternalInput")
    rwh_d = nc.dram_tensor("rwhi", [P, NDR, 2, RWPAD], _dt.float8e4, kind="ExternalInput")
    rwh16_d = nc.dram_tensor("rwh16", [P, NDR, 2, RWPAD], _dt.float8e4, kind="ExternalInput")
    rwl16_d = nc.dram_tensor("rwl16", [P, NDR, 2, RWPAD], _dt.float8e4, kind="ExternalInput")
    rb_d = nc.dram_tensor("rb", [P, E], _dt.float32, kind="ExternalInput")
    y_d = nc.dram_tensor("y", [bc, D], _dt.float32, kind="ExternalOutput")
    rt_ap = None
    if not skip_rt:
        rt_d = nc.dram_tensor("routing", [P, nt, E], _dt.float32, kind="ExternalOutput")
        rt_ap = rt_d.ap()

    x_ap, xhi_ap, xlo_ap = x_d.ap(), xhi_d.ap(), xlo_d.ap()
    w_ap, rwh_ap, rb_ap = w_d.ap(), rwh_d.ap(), rb_d.ap()
    rwh16_ap, rwl16_ap = rwh16_d.ap(), rwl16_d.ap()
    y_ap = y_d.ap()

    with tile.TileContext(nc) as tc:
        with (
            tc.tile_pool(name="const", bufs=1) as cpool,
            tc.tile_pool(name="xin", bufs=3) as xpool,
            tc.tile_pool(name="xh", bufs=4) as xhpool,
            tc.tile_pool(name="xl", bufs=4) as xlpool,
            tc.tile_pool(name="yout", bufs=2) as ypool,
            tc.tile_pool(name="tsc", bufs=4) as tpool,
            tc.tile_pool(name="small", bufs=4) as spool,
            tc.tile_pool(name="r", bufs=2, space="PSUM") as rpool,
            tc.tile_pool(name="z", bufs=6, space="PSUM") as zpool,
        ):
            # ---- tiny constants ----
            id32 = cpool.tile([P, P], _dt.float32, tag="id32")
            masks.make_identity(nc, id32[:])
            ident = cpool.tile([P, P], _dt.float32r, tag="ident")
            nc.vector.tensor_copy(ident[:], id32[:])
            eps_sb = cpool.tile([P, 1], _dt.float32, tag="eps")
            nc.vector.memset(eps_sb[:], float(EPS) * S_SCALE)

            # ---- PE warmup: open the HAM clock gate ----
            jpsum = zpool.tile([P, 512], _dt.float32, tag="z")
            for _ in range(40):
                nc.tensor.matmul(
                    jpsum[:, 0:128], ident[:], ident[:], start=True, stop=True
                )

            # ---- resident weights ----
            w_sb = cpool.tile([P, NDR, 2, D], _dt.float8e4, tag="w8")
            rwh_sb = cpool.tile([P, NDR, 2, RWPAD], _dt.float8e4, tag="rwhi")
            rwh16_sb = cpool.tile([P, NDR, 2, RWPAD], _dt.float8e4, tag="rwh16")
            rwl16_sb = cpool.tile([P, NDR, 2, RWPAD], _dt.float8e4, tag="rwl16")
            rb_sb = cpool.tile([P, E], _dt.float32, tag="rb")
            nc.sync.dma_start(rwh_sb[:], rwh_ap)
            nc.sync.dma_start(rwh16_sb[:], rwh16_ap)
            nc.sync.dma_start(rwl16_sb[:], rwl16_ap)
            nc.sync.dma_start(rb_sb[:], rb_ap)
            if not skip_rt:
                rt_sb = cpool.tile([P, nt, E], _dt.float32, tag="rt")

            # ---- prefetch first x tiles before the big W8 load ----
            prefetched = {}
            for i in range(min(2, nt)):
                xs = xpool.tile([P, D], _dt.float32, tag="x")
                nc.sync.dma_start(xs[:], x_ap[bass.ts(i, P), :])
                xh = xhpool.tile([P, KC, P], _dt.float8e4, tag="xh")
                nc.sync.dma_start(xh[:], xhi_ap[i, :, :])
                xl = xlpool.tile([P, KC, P], _dt.float8e4, tag="xl")
                nc.sync.dma_start(xl[:], xlo_ap[i, :, :])
                prefetched[i] = (xs, xh, xl)
            for c in range(NDR):
                nc.sync.dma_start(w_sb[:, c, :, :], w_ap[:, c, :, :])

            for i in range(nt):
                if i in prefetched:
                    xs, xh, xl = prefetched.pop(i)
                else:
                    xs = xpool.tile([P, D], _dt.float32, tag="x")
                    nc.sync.dma_start(xs[:], x_ap[bass.ts(i, P), :])
                    xh = xhpool.tile([P, KC, P], _dt.float8e4, tag="xh")
                    nc.sync.dma_start(xh[:], xhi_ap[i, :, :])
                    xl = xlpool.tile([P, KC, P], _dt.float8e4, tag="xl")
                    nc.sync.dma_start(xl[:], xlo_ap[i, :, :])

                y = ypool.tile([P, D], _dt.float32, tag="y")

                # ---- rms scale: rec = s * 2^-11, via ln/exp so the ACT
                # engine never swaps activation tables (ln+exp+square+copy
                # share one table; sqrt does not) ----
                ssq = spool.tile([P, 1], _dt.float32, tag="ssq")
                nc.scalar.activation(
                    y[:], xs[:], AF.Square, scale=float(D**-0.5), accum_out=ssq[:]
                )
                t = spool.tile([P, 1], _dt.float32, tag="t")
                nc.scalar.activation(t[:], ssq[:], AF.Ln, bias=eps_sb[:], scale=S_SCALE)
                rec = spool.tile([P, 1], _dt.float32, tag="rec")
                nc.scalar.activation(rec[:], t[:], AF.Exp, scale=-0.5)

                # ---- router, transposed: lhsT = rw pairs (tiny LDWEIGHTS),
                # moving = x pairs.  One psum group:
                #   lg^T = rwhi.xhi + (rwhi/16).xlo + (rwlo/16).xhi
                rA = rpool.tile([P, 512], _dt.float32, tag="r")
                chains = [(rwh_sb, xh), (rwh16_sb, xl), (rwl16_sb, xh)]
                for ci, (rw_sb, xq) in enumerate(chains):
                    for c in range(NDR):
                        nc.tensor.matmul(
                            rA[0:E, 0:P],
                            rw_sb[:, c, :, 0:E],
                            xq[:, 2 * c : 2 * c + 2, :],
                            start=(ci == 0 and c == 0),
                            stop=(ci == 2 and c == NDR - 1),
                            perf_mode=DR,
                        )

                # ---- transpose logits back to token-major ----
                lgT = spool.tile([E, P], _dt.float32, tag="lgT")
                nc.scalar.copy(lgT[:], rA[0:E, 0:P])
                lgtok = rpool.tile([P, 512], _dt.float32, tag="r")
                nc.tensor.transpose(lgtok[:, 0:E], lgT[:], id32[0:E, 0:E])

                # ---- softmax / coefficients (logits are ~N(0,1): no
                # max-subtraction needed for fp32 exp) ----
                exps = spool.tile([P, E], _dt.float32, tag="exps")
                se = spool.tile([P, 1], _dt.float32, tag="se")
                if rb_zero:
                    # exps = exp(raw * rec) in one ACT op
                    nc.scalar.activation(
                        exps[:], lgtok[:, 0:E], AF.Exp, scale=rec[:], accum_out=se[:]
                    )
                else:
                    logits = spool.tile([P, E], _dt.float32, tag="logits")
                    nc.vector.scalar_tensor_tensor(
                        logits[:],
                        lgtok[:, 0:E],
                        rec[:],
                        rb_sb[:],
                        op0=ALU.mult,
                        op1=ALU.add,
                    )
                    nc.scalar.activation(
                        exps[:], logits[:], AF.Exp, scale=1.0, accum_out=se[:]
                    )
                r2 = spool.tile([P, 1], _dt.float32, tag="r2")
                nc.vector.reciprocal(r2[:], se[:])
                cs = spool.tile([P, 1], _dt.float32, tag="cs")
                nc.vector.tensor_mul(cs[:], r2[:], rec[:])
                coef = spool.tile([P, E], _dt.float32, tag="coef")
                nc.vector.tensor_scalar_mul(coef[:], exps[:], cs[:])
                if not skip_rt:
                    nc.vector.tensor_scalar_mul(rt_sb[:, i, :], exps[:], r2[:])

                # ---- main GEMM (fp8 DR) + combine ----
                for j in range(NJ):
                    zs = [
                        zpool.tile([P, 512], _dt.float32, tag="z", name=f"z{j}_{e}")
                        for e in range(E)
                    ]
                    for e in range(E):
                        for c2 in range(2):
                            c = 2 * e + c2
                            nc.tensor.matmul(
                                zs[e][:],
                                xh[:, 2 * c : 2 * c + 2, :],
                                w_sb[:, c, :, bass.ts(j, 512)],
                                start=(c2 == 0),
                                stop=(c2 == 1),
                                perf_mode=DR,
                            )
                    if j < 2:
                        # DVE combine chain (reads psum directly)
                        for e in range(E):
                            in1 = (
                                xs[:, bass.ts(j, 512)]
                                if e == 0
                                else y[:, bass.ts(j, 512)]
                            )
                            nc.vector.scalar_tensor_tensor(
                                y[:, bass.ts(j, 512)],
                                zs[e][:],
                                coef[:, e : e + 1],
                                in1,
                                op0=ALU.mult,
                                op1=ALU.add,
                            )
                    else:
                        # ACT drains coef-scaled z to bf16; DVE sums with a
                        # 2x-throughput bf16 tree, final mixed add with x
                        ts_ = tpool.tile(
                            [P, E, 512], _dt.bfloat16, tag="ts", name=f"ts{j}"
                        )
                        for e in range(E):
                            nc.scalar.activation(
                                ts_[:, e, :],
                                zs[e][:],
                                AF.Copy,
                                scale=coef[:, e : e + 1],
                            )
                        with nc.allow_low_precision("bf16 combine partials"):
                            u1 = tpool.tile(
                                [P, 512], _dt.bfloat16, tag="u1", name=f"u1_{j}"
                            )
                            nc.vector.tensor_add(u1[:], ts_[:, 0, :], ts_[:, 1, :])
                            u2 = tpool.tile(
                                [P, 512], _dt.bfloat16, tag="u2", name=f"u2_{j}"
                            )
                            nc.vector.tensor_add(u2[:], ts_[:, 2, :], ts_[:, 3, :])
                            u3 = tpool.tile(
                                [P, 512], _dt.bfloat16, tag="u3", name=f"u3_{j}"
                            )
                            nc.vector.tensor_add(u3[:], u1[:], u2[:])
                        nc.vector.tensor_add(
                            y[:, bass.ts(j, 512)], u3[:], xs[:, bass.ts(j, 512)]
                        )
                nc.sync.dma_start(y_ap[bass.ts(i, P), :], y[:])

            if not skip_rt:
                nc.sync.dma_start(rt_ap, rt_sb[:])

    nc.compile()
    return nc


_built = {}


def _get_nc(nt: int, skip_rt: bool, rb_zero: bool):
    key = (nt, skip_rt, rb_zero)
    if key not in _built:
        _built[key] = build(nt, skip_rt=skip_rt, rb_zero=rb_zero)
    return _built[key]


def prepare_weights(norm_w, router_w, router_b, qkv_w, proj_w, proj_b, out_w):
    """Host-side fold of all linear stages into per-expert [512, 2048] mats."""
    nw = norm_w.astype(np.float64)
    Wv = qkv_w[:, :, 2 * dE :].astype(np.float64)  # [E, 512, 512]
    pw = proj_w.astype(np.float64)
    ow = out_w.astype(np.float64)
    W = np.empty((D, D), dtype=np.float64)  # folded, feature-major rows
    C = np.empty((E, D), dtype=np.float64)
    for e in range(E):
        nw_e = nw[e * dE : (e + 1) * dE]
        ow_e = ow[e * dE : (e + 1) * dE, :]  # [512, 2048]
        W[e * dE : (e + 1) * dE] = (nw_e[:, None] * Wv[e]) @ pw[e] @ ow_e
        C[e] = proj_b[e].astype(np.float64) @ ow_e
    rw_fold = nw[:, None] * router_w.astype(np.float64)  # [D, E]

    # quantize: W*2^7 -> fp8; rw hi/lo with the 1/16 pre-folded so all three
    # router chains share one psum accumulation group
    W8 = (W * SW).astype(np.float32).astype(F8)  # [D, D]
    rwf = (rw_fold * SW).astype(np.float32)
    rw_hi = rwf.astype(F8)
    rw_lo = ((rwf - rw_hi.astype(np.float32)) * np.float32(16)).astype(F8)
    rw_h16 = (rw_hi.astype(np.float32) / np.float32(16)).astype(F8)
    rw_l16 = (rw_lo.astype(np.float32) / np.float32(16)).astype(F8)

    # device layouts
    w_dev = np.ascontiguousarray(
        W8.reshape(NDR, 2, P, D).transpose(2, 0, 1, 3)
    )  # [P, NDR, 2, D]

    def rw_dev(r8):
        out = np.zeros((P, NDR, 2, RWPAD), dtype=F8)
        out[:, :, :, 0:E] = r8.reshape(NDR, 2, P, E).transpose(2, 0, 1, 3)
        return np.ascontiguousarray(out)

    rb_dev = np.tile(router_b.astype(np.float32)[None, :], (P, 1))
    return w_dev, rw_dev(rw_hi), rw_dev(rw_h16), rw_dev(rw_l16), rb_dev, C


def prepare_x(x):
    """Quantize + transpose x on host: tile-blocked feature-major hi/lo fp8."""
    xs = x * np.float32(SX)
    x_hi = xs.astype(F8)
    x_lo = ((xs - x_hi.astype(np.float32)) * np.float32(16)).astype(F8)

    def blk(q):
        # [B, D] -> per-core [NT, P(feat), D(=KC*128 tokens-major? no:)]
        # layout: dev[core][i, p, k*128+m] = q[core*BC + i*128 + m, k*128 + p]
        a = q.reshape(N_CORES, NT, P, KC, P)  # [core, i, m, k, p]
        return np.ascontiguousarray(a.transpose(0, 1, 4, 3, 2)).reshape(
            N_CORES, NT, P, D
        )

    return blk(x_hi), blk(x_lo)


def _ensure_ntff_hook():
    """Make NTFF profiling work: antenv in the image lacks axon_hooks."""
    import types as _types

    try:
        from antenv.axon_hooks import get_axon_ntff_profile_hook  # noqa: F401
    except ImportError:
        import antenv

        mod = _types.ModuleType("antenv.axon_hooks")
        _hook = [None]
        mod.set_axon_ntff_profile_hook = lambda h: _hook.__setitem__(0, h)
        mod.get_axon_ntff_profile_hook = lambda: _hook[0]
        sys.modules["antenv.axon_hooks"] = mod
        antenv.axon_hooks = mod
    from antenv import axon_hooks

    if axon_hooks.get_axon_ntff_profile_hook() is None:
        from trn_agent_boot.trn_boot import _ntff_profile_via_ctypes

        h = _ntff_profile_via_ctypes("/opt/axon/libaxon_pjrt.so")
        if h is not None:
            axon_hooks.set_axon_ntff_profile_hook(h)


def kernel(x, norm_w, router_w, router_b, qkv_w, proj_w, proj_b, out_w, _trace=False):
    if _trace:
        try:
            _ensure_ntff_hook()
        except Exception as e:  # profiling is best-effort
            print("ntff hook setup failed:", e)
    x = np.ascontiguousarray(np.asarray(x, dtype=np.float32))
    w_dev, rwh_dev, rwh16_dev, rwl16_dev, rb_dev, C = prepare_weights(
        np.asarray(norm_w),
        np.asarray(router_w),
        np.asarray(router_b),
        np.asarray(qkv_w),
        np.asarray(proj_w),
        np.asarray(proj_b),
        np.asarray(out_w),
    )
    xhi_dev, xlo_dev = prepare_x(x)
    skip_rt = bool(np.all(np.asarray(proj_b) == 0.0))
    rb_zero = bool(np.all(np.asarray(router_b) == 0.0))
    nc = _get_nc(NT, skip_rt, rb_zero)
    in_maps = []
    for c in range(N_CORES):
        in_maps.append(
            {
                "x": x[c * BC : (c + 1) * BC],
                "xhi": xhi_dev[c].reshape(NT, P, D),
                "xlo": xlo_dev[c].reshape(NT, P, D),
                "w8": w_dev,
                "rwhi": rwh_dev,
                "rwh16": rwh16_dev,
                "rwl16": rwl16_dev,
                "rb": rb_dev,
            }
        )
    res = bass_utils.run_bass_kernel_spmd(
        nc, in_maps, core_ids=list(range(N_CORES)), trace=_trace
    )
    y = np.concatenate([res.results[c]["y"] for c in range(N_CORES)], axis=0)
    if not skip_rt and np.any(C != 0.0):
        # routing dram layout [P, NT, E]: token b = i*128 + m -> rt[m, i, e]
        routing = np.concatenate(
            [
                res.results[c]["routing"].transpose(1, 0, 2).reshape(BC, E)
                for c in range(N_CORES)
            ],
            axis=0,
        )
        y = (y.astype(np.float64) + routing.astype(np.float64) @ C).astype(np.float32)
    if _trace:
        kernel._last_results = res
    return y


# revision 26
# speedup vs baseline: 1.1908x; 1.1908x over previous
"""MixtureOfAttention forward for Trainium2 (8 NeuronCores, data-parallel over B).

Math (exactly equivalent to the reference):
  s_b   = rsqrt(mean(x_b^2) + eps)                      (per token)
  logits= s * (x @ (diag(norm_w) @ router_w)) + router_b
  r     = softmax(logits)                                [B, 4]
  y     = x + sum_e (r_e * s) * (x_e @ W_e) + r @ C
  W_e   = diag(norm_w_e) @ Wv_e @ proj_w_e @ out_w_e     [512, 2048]  (host-folded)
  C_e   = proj_b_e @ out_w_e                             [2048]       (host-folded)
(The seq_len==1 attention is the identity on v, so only the v-slice of qkv_w
participates.  The r @ C term is applied on host from the device-computed
routing probs; it is exactly zero for proj_b == 0.)

Implementation: all GEMMs run in fp8-e4m3 with DoubleRow perf mode
(contraction 256/matmul, ~1.8x fp32r throughput).  x is pre-transposed and
pre-quantized on host (hi + lo residual); the router consumes hi/lo for
near-exact logits, the main GEMM consumes hi only.  Scales: x*2^4, W*2^7,
residuals *2^4 further; the combined 2^-11 is folded into the rms scale s.
"""

import sys

sys.path.insert(0, "/opt/trn_rl_repo")

import numpy as np
import ml_dtypes

import concourse.bass as bass
import concourse.bacc as bacc
import concourse.mybir as mybir
import concourse.tile as tile
from concourse import bass_utils, masks

F8 = ml_dtypes.float8_e4m3

B, D, E = 32768, 2048, 4
dE = D // E  # 512
EPS = 1e-6
N_CORES = 8
P = 128
BC = B // N_CORES  # 4096 tokens per core
NT = BC // P  # 32 tiles per core
KC = D // P  # 16 feature chunks
NDR = KC // 2  # 8 DoubleRow chunks (256 features each)
NJ = 4  # output chunks of 512
RWPAD = 16  # router weight free-dim pad (16B alignment for the DR pair stride)

SX = 16.0  # x scale before fp8
SW = 128.0  # W / router_w scale before fp8
S_SCALE = float(2.0**22)  # folds 2^-11 (=1/(SX*SW)) into rec = 1/sqrt(...)

_dt = mybir.dt
AF = mybir.ActivationFunctionType
ALU = mybir.AluOpType
DR = mybir.MatmulPerfMode.DoubleRow


def build(nt: int, skip_rt: bool = False, rb_zero: bool = False):
    bc = nt * P
    nc = bacc.Bacc("TRN2", target_bir_lowering=False, debug=False, num_devices=N_CORES)

    x_d = nc.dram_tensor("x", [bc, D], _dt.float32, kind="ExternalInput")
    xhi_d = nc.dram_tensor("xhi", [nt, P, D], _dt.float8e4, kind="ExternalInput")
    xlo_d = nc.dram_tensor("xlo", [nt, P, D], _dt.float8e4, kind="ExternalInput")
    w_d = nc.dram_tensor("w8", [P, NDR, 2, D], _dt.float8e4, kind="ExternalInput")
    rwh_d = nc.dram_tensor("rwhi", [P, NDR, 2, RWPAD], _dt.float8e4, kind="ExternalInput")
    rwh16_d = nc.dram_tensor("rwh16", [P, NDR, 2, RWPAD], _dt.float8e4, kind="ExternalInput")
    rwl16_d = nc.dram_tensor("rwl16", [P, NDR, 2, RWPAD], _dt.float8e4, kind="ExternalInput")
    rb_d = nc.dram_tensor("rb", [P, E], _dt.float32, kind="ExternalInput")
    y_d = nc.dram_tensor("y", [bc, D], _dt.float32, kind="ExternalOutput")
    rt_ap = None
    if not skip_rt:
        rt_d = nc.dram_tensor("routing", [P, nt, E], _dt.float32, kind="ExternalOutput")
        rt_ap = rt_d.ap()

    x_ap, xhi_ap, xlo_ap = x_d.ap(), xhi_d.ap(), xlo_d.ap()
    w_ap, rwh_ap, rb_ap = w_d.ap(), rwh_d.ap(), rb_d.ap()
    rwh16_ap, rwl16_ap = rwh16_d.ap(), rwl16_d.ap()
    y_ap = y_d.ap()

    with tile.TileContext(nc) as tc:
        with (
            tc.tile_pool(name="const", bufs=1) as cpool,
            tc.tile_pool(name="xin", bufs=3) as xpool,
            tc.tile_pool(name="xh", bufs=4) as xhpool,
            tc.tile_pool(name="xl", bufs=4) as xlpool,
            tc.tile_pool(name="yout", bufs=2) as ypool,
            tc.tile_pool(name="tsc", bufs=4) as tpool,
            tc.tile_pool(name="small", bufs=4) as spool,
            tc.tile_pool(name="r", bufs=2, space="PSUM") as rpool,
            tc.tile_pool(name="z", bufs=6, space="PSUM") as zpool,
        ):
            # ---- tiny constants ----
            id32 = cpool.tile([P, P], _dt.float32, tag="id32")
            masks.make_identity(nc, id32[:])
            ident = cpool.tile([P, P], _dt.float32r, tag="ident")
            nc.vector.tensor_copy(ident[:], id32[:])
            eps_sb = cpool.tile([P, 1], _dt.float32, tag="eps")
            nc.vector.memset(eps_sb[:], float(EPS) * S_SCALE)

            # ---- PE warmup: open the HAM clock gate ----
            jpsum = zpool.tile([P, 512], _dt.float32, tag="z")
            for _ in range(40):
                nc.tensor.matmul(
                    jpsum[:, 0:128], ident[:], ident[:], start=True, stop=True
                )

            # ---- resident weights ----
            w_sb = cpool.tile([P, NDR, 2, D], _dt.float8e4, tag="w8")
            rwh_sb = cpool.tile([P, NDR, 2, RWPAD], _dt.float8e4, tag="rwhi")
            rwh16_sb = cpool.tile([P, NDR, 2, RWPAD], _dt.float8e4, tag="rwh16")
            rwl16_sb = cpool.tile([P, NDR, 2, RWPAD], _dt.float8e4, tag="rwl16")
            rb_sb = cpool.tile([P, E], _dt.float32, tag="rb")
            nc.sync.dma_start(rwh_sb[:], rwh_ap)
            nc.sync.dma_start(rwh16_sb[:], rwh16_ap)
            nc.sync.dma_start(rwl16_sb[:], rwl16_ap)
            nc.sync.dma_start(rb_sb[:], rb_ap)
            if not skip_rt:
                rt_sb = cpool.tile([P, nt, E], _dt.float32, tag="rt")

            # ---- prefetch first x tiles before the big W8 load ----
            prefetched = {}
            for i in range(min(2, nt)):
                xs = xpool.tile([P, D], _dt.float32, tag="x")
                nc.sync.dma_start(xs[:], x_ap[bass.ts(i, P), :])
                xh = xhpool.tile([P, KC, P], _dt.float8e4, tag="xh")
                nc.sync.dma_start(xh[:], xhi_ap[i, :, :])
                xl = xlpool.tile([P, KC, P], _dt.float8e4, tag="xl")
                nc.sync.dma_start(xl[:], xlo_ap[i, :, :])
                prefetched[i] = (xs, xh, xl)
            for c in range(NDR):
                nc.sync.dma_start(w_sb[:, c, :, :], w_ap[:, c, :, :])

            for i in range(nt):
                if i in prefetched:
                    xs, xh, xl = prefetched.pop(i)
                else:
                    xs = xpool.tile([P, D], _dt.float32, tag="x")
                    nc.sync.dma_start(xs[:], x_ap[bass.ts(i, P), :])
                    xh = xhpool.tile([P, KC, P], _dt.float8e4, tag="xh")
                    nc.sync.dma_start(xh[:], xhi_ap[i, :, :])
                    xl = xlpool.tile([P, KC, P], _dt.float8e4, tag="xl")
                    nc.sync.dma_start(xl[:], xlo_ap[i, :, :])

                y = ypool.tile([P, D], _dt.float32, tag="y")

                # ---- rms scale: rec = s * 2^-11, via ln/exp so the ACT
                # engine never swaps activation tables (ln+exp+square+copy
                # share one table; sqrt does not) ----
                ssq = spool.tile([P, 1], _dt.float32, tag="ssq")
                nc.scalar.activation(
                    y[:], xs[:], AF.Square, scale=float(D**-0.5), accum_out=ssq[:]
                )
                t = spool.tile([P, 1], _dt.float32, tag="t")
                nc.scalar.activation(t[:], ssq[:], AF.Ln, bias=eps_sb[:], scale=S_SCALE)
                rec = spool.tile([P, 1], _dt.float32, tag="rec")
                nc.scalar.activation(rec[:], t[:], AF.Exp, scale=-0.5)

                # ---- router, transposed: lhsT = rw pairs (tiny LDWEIGHTS),
                # moving = x pairs.  One psum group:
                #   lg^T = rwhi.xhi + (rwhi/16).xlo + (rwlo/16).xhi
                rA = rpool.tile([P, 512], _dt.float32, tag="r")
                chains = [(rwh_sb, xh), (rwh16_sb, xl), (rwl16_sb, xh)]
                for ci, (rw_sb, xq) in enumerate(chains):
                    for c in range(NDR):
                        nc.tensor.matmul(
                            rA[0:E, 0:P],
                            rw_sb[:, c, :, 0:E],
                            xq[:, 2 * c : 2 * c + 2, :],
                            start=(ci == 0 and c == 0),
                            stop=(ci == 2 and c == NDR - 1),
                            perf_mode=DR,
                        )

                # ---- transpose logits back to token-major ----
                lgT = spool.tile([E, P], _dt.float32, tag="lgT")
                nc.scalar.copy(lgT[:], rA[0:E, 0:P])
                lgtok = rpool.tile([P, 512], _dt.float32, tag="r")
                nc.tensor.transpose(lgtok[:, 0:E], lgT[:], id32[0:E, 0:E])

                # ---- softmax / coefficients (logits are ~N(0,1): no
                # max-subtraction needed for fp32 exp) ----
                exps = spool.tile([P, E], _dt.float32, tag="exps")
                se = spool.tile([P, 1], _dt.float32, tag="se")
                if rb_zero:
                    # exps = exp(raw * rec) in one ACT op
                    nc.scalar.activation(
                        exps[:], lgtok[:, 0:E], AF.Exp, scale=rec[:], accum_out=se[:]
                    )
                else:
                    logits = spool.tile([P, E], _dt.float32, tag="logits")
                    nc.vector.scalar_tensor_tensor(
                        logits[:],
                        lgtok[:, 0:E],
                        rec[:],
                        rb_sb[:],
                        op0=ALU.mult,
                        op1=ALU.add,
                    )
                    nc.scalar.activation(
                        exps[:], logits[:], AF.Exp, scale=1.0, accum_out=se[:]
                    )
                r2 = spool.tile([P, 1], _dt.float32, tag="r2")
                nc.vector.reciprocal(r2[:], se[:])
                cs = spool.tile([P, 1], _dt.float32, tag="cs")
                nc.vector.tensor_mul(cs[:], r2[:], rec[:])
                coef = spool.tile([P, E], _dt.float32, tag="coef")
                nc.vector.tensor_scalar_mul(coef[:], exps[:], cs[:])
                if not skip_rt:
                    nc.vector.tensor_scalar_mul(rt_sb[:, i, :], exps[:], r2[:])

                # ---- main GEMM (fp8 DR) + combine ----
                for j in range(NJ):
                    zs = [
                        zpool.tile([P, 512], _dt.float32, tag="z", name=f"z{j}_{e}")
                        for e in range(E)
                    ]
                    for e in range(E):
                        for c2 in range(2):
                            c = 2 * e + c2
                            nc.tensor.matmul(
                                zs[e][:],
                                xh[:, 2 * c : 2 * c + 2, :],
                                w_sb[:, c, :, bass.ts(j, 512)],
                                start=(c2 == 0),
                                stop=(c2 == 1),
                                perf_mode=DR,
                            )
                    for e in range(E):
                        in1 = (
                            xs[:, bass.ts(j, 512)] if e == 0 else y[:, bass.ts(j, 512)]
                        )
                        nc.vector.scalar_tensor_tensor(
                            y[:, bass.ts(j, 512)],
                            zs[e][:],
                            coef[:, e : e + 1],
                            in1,
                            op0=ALU.mult,
                            op1=ALU.add,
                        )
                nc.sync.dma_start(y_ap[bass.ts(i, P), :], y[:])

            if not skip_rt:
                nc.sync.dma_start(rt_ap, rt_sb[:])

    nc.compile()
    return nc


_built = {}


def _get_nc(nt: int, skip_rt: bool, rb_zero: bool):
    key = (nt, skip_rt, rb_zero)
    if key not in _built:
        _built[key] = build(nt, skip_rt=skip_rt, rb_zero=rb_zero)
    return _built[key]


def prepare_weights(norm_w, router_w, router_b, qkv_w, proj_w, proj_b, out_w):
    """Host-side fold of all linear stages into per-expert [512, 2048] mats."""
    nw = norm_w.astype(np.float64)
    Wv = qkv_w[:, :, 2 * dE :].astype(np.float64)  # [E, 512, 512]
    pw = proj_w.astype(np.float64)
    ow = out_w.astype(np.float64)
    W = np.empty((D, D), dtype=np.float64)  # folded, feature-major rows
    C = np.empty((E, D), dtype=np.float64)
    for e in range(E):
        nw_e = nw[e * dE : (e + 1) * dE]
        ow_e = ow[e * dE : (e + 1) * dE, :]  # [512, 2048]
        W[e * dE : (e + 1) * dE] = (nw_e[:, None] * Wv[e]) @ pw[e] @ ow_e
        C[e] = proj_b[e].astype(np.float64) @ ow_e
    rw_fold = nw[:, None] * router_w.astype(np.float64)  # [D, E]

    # quantize: W*2^7 -> fp8; rw hi/lo with the 1/16 pre-folded so all three
    # router chains share one psum accumulation group
    W8 = (W * SW).astype(np.float32).astype(F8)  # [D, D]
    rwf = (rw_fold * SW).astype(np.float32)
    rw_hi = rwf.astype(F8)
    rw_lo = ((rwf - rw_hi.astype(np.float32)) * np.float32(16)).astype(F8)
    rw_h16 = (rw_hi.astype(np.float32) / np.float32(16)).astype(F8)
    rw_l16 = (rw_lo.astype(np.float32) / np.float32(16)).astype(F8)

    # device layouts
    w_dev = np.ascontiguousarray(
        W8.reshape(NDR, 2, P, D).transpose(2, 0, 1, 3)
    )  # [P, NDR, 2, D]

    def rw_dev(r8):
        out = np.zeros((P, NDR, 2, RWPAD), dtype=F8)
        out[:, :, :, 0:E] = r8.reshape(NDR, 2, P, E).transpose(2, 0, 1, 3)
        return np.ascontiguousarray(out)

    rb_dev = np.tile(router_b.astype(np.float32)[None, :], (P, 1))
    return w_dev, rw_dev(rw_hi), rw_dev(rw_h16), rw_dev(rw_l16), rb_dev, C


def prepare_x(x):
    """Quantize + transpose x on host: tile-blocked feature-major hi/lo fp8."""
    xs = x * np.float32(SX)
    x_hi = xs.astype(F8)
    x_lo = ((xs - x_hi.astype(np.float32)) * np.float32(16)).astype(F8)

    def blk(q):
        # [B, D] -> per-core [NT, P(feat), D(=KC*128 tokens-major? no:)]
        # layout: dev[core][i, p, k*128+m] = q[core*BC + i*128 + m, k*128 + p]
        a = q.reshape(N_CORES, NT, P, KC, P)  # [core, i, m, k, p]
        return np.ascontiguousarray(a.transpose(0, 1, 4, 3, 2)).reshape(
            N_CORES, NT, P, D
        )

    return blk(x_hi), blk(x_lo)


def _ensure_ntff_hook():
    """Make NTFF profiling work: antenv in the image lacks axon_hooks."""
    import types as _types

    try:
        from antenv.axon_hooks import get_axon_ntff_profile_hook  # noqa: F401
    except ImportError:
        import antenv

        mod = _types.ModuleType("antenv.axon_hooks")
        _hook = [None]
        mod.set_axon_ntff_profile_hook = lambda h: _hook.__setitem__(0, h)
        mod.get_axon_ntff_profile_hook = lambda: _hook[0]
        sys.modules["antenv.axon_hooks"] = mod
        antenv.axon_hooks = mod
    from antenv import axon_hooks

    if axon_hooks.get_axon_ntff_profile_hook() is None:
        from trn_agent_boot.trn_boot import _ntff_profile_via_ctypes

        h = _ntff_profile_via_ctypes("/opt/axon/libaxon_pjrt.so")
        if h is not None:
            axon_hooks.set_axon_ntff_profile_hook(h)


def kernel(x, norm_w, router_w, router_b, qkv_w, proj_w, proj_b, out_w, _trace=False):
    if _trace:
        try:
            _ensure_ntff_hook()
        except Exception as e:  # profiling is best-effort
            print("ntff hook setup failed:", e)
    x = np.ascontiguousarray(np.asarray(x, dtype=np.float32))
    w_dev, rwh_dev, rwh16_dev, rwl16_dev, rb_dev, C = prepare_weights(
        np.asarray(norm_w),
        np.asarray(router_w),
        np.asarray(router_b),
        np.asarray(qkv_w),
        np.asarray(proj_w),
        np.asarray(proj_b),
        np.asarray(out_w),
    )
    xhi_dev, xlo_dev = prepare_x(x)
    skip_rt = bool(np.all(np.asarray(proj_b) == 0.0))
    rb_zero = bool(np.all(np.asarray(router_b) == 0.0))
    nc = _get_nc(NT, skip_rt, rb_zero)
    in_maps = []
    for c in range(N_CORES):
        in_maps.append(
            {
                "x": x[c * BC : (c + 1) * BC],
                "xhi": xhi_dev[c].reshape(NT, P, D),
                "xlo": xlo_dev[c].reshape(NT, P, D),
                "w8": w_dev,
                "rwhi": rwh_dev,
                "rwh16": rwh16_dev,
                "rwl16": rwl16_dev,
                "rb": rb_dev,
            }
        )
    res = bass_utils.run_bass_kernel_spmd(
        nc, in_maps, core_ids=list(range(N_CORES)), trace=_trace
    )
    y = np.concatenate([res.results[c]["y"] for c in range(N_CORES)], axis=0)
    if not skip_rt and np.any(C != 0.0):
        # routing dram layout [P, NT, E]: token b = i*128 + m -> rt[m, i, e]
        routing = np.concatenate(
            [
                res.results[c]["routing"].transpose(1, 0, 2).reshape(BC, E)
                for c in range(N_CORES)
            ],
            axis=0,
        )
        y = (y.astype(np.float64) + routing.astype(np.float64) @ C).astype(np.float32)
    if _trace:
        kernel._last_results = res
    return y
